# revision 28
# baseline (speedup 1.0000x reference)
"""Deep-Koopman-operator kernel for 8 Trainium2 NeuronCores.

Math: z0 = MLP(x[:, O-1]); for t in [0,L): z <- A z + B u_t ; y_t = C z ;
loss = mean_t mean_{B,S} (y_t - x_tgt_t)^2.  Returns (loss, ys[L,B,S]).

Strategy: data-parallel over batch (B=256 -> 32 per core).  The linear
recurrence is reformulated as a chunked scan with chunk length K:
  boundary states:  Z_{c+1} = A^K Z_c + M @ u_chunk_c      (small, sequential)
  outputs:          y_{cK+j} = (C A^j) Z_c + sum_i (C A^{j-i} B) u_{cK+i}
which turns almost all work into large matmuls (P = stacked C A^j,
T = block-Toeplitz of C A^d B).  Operators are precomputed on host in f64
from the passed-in A/B/C params (param preprocessing, like weight layout
transforms).  Matmuls run in float32r (full-rate fp32 mode, ~1e-4 rounding);
measured end-to-end scale-relative error vs the f32 reference is ~3e-3.

The reference recurrence diverges (spectral radius of A ~ 1.15), so f32
overflows around t~600 and the reference output has an inf/NaN tail.  The
device computes everything; the host then re-runs only the overflow
transition window with the exact sequential-f32 semantics of the reference
so the NaN onset pattern matches, and fills NaN beyond (which is what the
sequential recurrence produces once the state is non-finite).
"""

import os

import numpy as np

import concourse.bacc as bacc
import concourse.mybir as mybir
import concourse.tile as tile
from concourse.bass_utils import run_bass_kernel_spmd

# Problem dims (hardcoded per contract)
B_SZ, O, L, N, S, HID, UP = 256, 8, 1024, 256, 32, 512, 16
NCORES = 8
BLOC = B_SZ // NCORES  # 32 batch rows per core
K = 32                 # chunk length
CN = L // K            # 32 chunks
KUP = K * UP           # 512
KS = K * S             # 1024
CB = CN * BLOC         # 1024 streaming columns (chunk-major, batch-minor)
F32 = mybir.dt.float32
F32R = mybir.dt.float32r

N_T = N // 128         # 2
KUP_T = KUP // 128     # 4
KS_T = KS // 128       # 8
HID_T = HID // 128     # 4
HALF = 512             # stream split (one PSUM bank of f32)
NHALF = CB // HALF     # 2

USE_F32R = os.environ.get("DKO_F32R", "1") == "1"
DT_OP = F32R if USE_F32R else F32  # dtype of matmul operand tensors
TRACE = os.environ.get("DKO_TRACE", "0") == "1"

# keep results of the last run for test harness inspection
LAST_RESULT = None


def _conv_tile_nonzero(ct, r):
    # T.T tile (ct, r) nonzero iff exists j >= i with j in [4r+1,4r+4],
    # i in [8ct+1, 8ct+8]  <=>  4r+4 >= 8ct+1
    return 4 * r + 4 >= 8 * ct + 1


def _precompute_operators(A, Bm, Cm):
    """Host f64 precompute of chunk operators, returned as f32 lhsT layouts."""
    A64 = A.astype(np.float64)
    B64 = Bm.astype(np.float64)
    C64 = Cm.astype(np.float64)
    Apow = [np.eye(N)]
    for _ in range(K):
        Apow.append(A64 @ Apow[-1])
    # P = stack_j C A^j (j=1..K)  [K*S, N]
    P = np.concatenate([C64 @ Apow[j] for j in range(1, K + 1)], axis=0)
    Ak = Apow[K]
    # M = [A^{K-1} B, ..., A^0 B]  [N, K*UP]
    M = np.concatenate([Apow[K - 1 - i] @ B64 for i in range(K)], axis=1)
    # T block lower triangular of Q_d = C A^d B  [K*S, K*UP]
    Q = [C64 @ Apow[d] @ B64 for d in range(K)]
    T = np.zeros((KS, KUP), np.float64)
    for j in range(1, K + 1):
        for i in range(1, j + 1):
            T[(j - 1) * S:j * S, (i - 1) * UP:i * UP] = Q[j - i]
    Ak2 = Ak @ Ak
    Ak4 = Ak2 @ Ak2
    out = {
        "akt": np.ascontiguousarray(
            Ak.T.astype(np.float32).reshape(N_T, 128, N)),
        "ak2t": np.ascontiguousarray(
            Ak2.T.astype(np.float32).reshape(N_T, 128, N)),
        "ak4t": np.ascontiguousarray(
            Ak4.T.astype(np.float32).reshape(N_T, 128, N)),
        "mt": np.ascontiguousarray(
            M.T.astype(np.float32).reshape(KUP_T, 128, N)),
        "pt": np.ascontiguousarray(
            P.T.astype(np.float32).reshape(N_T, 128, KS)),
        "tt": np.ascontiguousarray(
            T.T.astype(np.float32).reshape(KUP_T, 128, KS)),
    }
    return out


def _build_program():
    """Build the Bass/Tile SPMD program (same program on all 8 cores)."""
    nc = bacc.Bacc(trn_type="TRN2", target_bir_lowering=False, debug=False,
                   num_devices=NCORES)

    d = {}
    d["ut0"] = nc.dram_tensor("ut0", [128, CB], DT_OP, kind="ExternalInput")
    d["mtp"] = nc.dram_tensor("mtp", [128, 4 * N], DT_OP,
                              kind="ExternalInput")
    d["prm"] = nc.dram_tensor("prm", [128, 10 * N], DT_OP,
                              kind="ExternalInput")
    d["w0t"] = nc.dram_tensor("w0t", [S, HID], DT_OP, kind="ExternalInput")
    d["xo"] = nc.dram_tensor("xo", [S, BLOC], DT_OP, kind="ExternalInput")
    d["bb"] = nc.dram_tensor("bb", [128, HID_T + N_T], F32,
                             kind="ExternalInput")
    d["ut123"] = nc.dram_tensor("ut123", [128, 3 * CB], DT_OP,
                                kind="ExternalInput")
    d["tp"] = nc.dram_tensor("tp", [128, 6 * KS], DT_OP,
                             kind="ExternalInput")
    d["xta"] = nc.dram_tensor("xta", [128, 4 * CB], F32,
                              kind="ExternalInput")
    d["xtb"] = nc.dram_tensor("xtb", [128, 4 * CB], F32,
                              kind="ExternalInput")
    d["yst"] = nc.dram_tensor("yst", [KS_T, 128, CB], F32,
                              kind="ExternalOutput")
    d["zat"] = nc.dram_tensor("zat", [128, N_T * CB], DT_OP,
                              kind="ExternalOutput")
    d["sse"] = nc.dram_tensor("sse", [128, KS_T * NHALF], F32,
                              kind="ExternalOutput")

    ap = {k: v.ap() for k, v in d.items()}

    with tile.TileContext(nc) as tc:
        from contextlib import ExitStack
        with ExitStack() as ctx:
            consts = ctx.enter_context(tc.tile_pool(name="consts", bufs=1))
            work = ctx.enter_context(tc.tile_pool(name="work", bufs=3))
            ps_b = ctx.enter_context(
                tc.tile_pool(name="ps_b", bufs=2, space="PSUM"))
            ps_y = ctx.enter_context(
                tc.tile_pool(name="ps_y", bufs=6, space="PSUM"))

            # ---- persistent SBUF tiles + input DMAs (few, large) ----
            mtp_s = consts.tile([128, 4 * N], DT_OP, tag="mtp", name="mtp")
            nc.sync.dma_start(mtp_s[:], ap["mtp"][:])
            ut0_s = consts.tile([128, CB], DT_OP, tag="ut0", name="ut0")
            nc.sync.dma_start(ut0_s[:], ap["ut0"][:])
            w0t_s = consts.tile([S, HID], DT_OP, tag="w0t", name="w0t_s")
            nc.sync.dma_start(w0t_s[:], ap["w0t"][:])
            xo_s = consts.tile([S, BLOC], DT_OP, tag="xo", name="xo")
            nc.sync.dma_start(xo_s[:], ap["xo"][:])
            bb_s = consts.tile([128, HID_T + N_T], F32, tag="bb", name="bb")
            nc.sync.dma_start(bb_s[:], ap["bb"][:])
            ut123_s = consts.tile([128, 3 * CB], DT_OP, tag="ut123",
                                  name="ut123")
            nc.scalar.dma_start(ut123_s[:], ap["ut123"][:])
            prm_s = consts.tile([128, 10 * N], DT_OP, tag="prm", name="prm")
            nc.sync.dma_start(prm_s[:], ap["prm"][:])
            tp_s = consts.tile([128, 6 * KS], DT_OP, tag="tp", name="tp")
            nc.scalar.dma_start(tp_s[:], ap["tp"][:])
            xta_s = consts.tile([128, 4 * CB], F32, tag="xta", name="xta")
            nc.scalar.dma_start(xta_s[:], ap["xta"][:])
            xtb_s = consts.tile([128, 4 * CB], F32, tag="xtb", name="xtb")
            nc.sync.dma_start(xtb_s[:], ap["xtb"][:])

            # slice views matching the old per-tile layout
            def ut_sl(ct):
                return (ut0_s[:] if ct == 0
                        else ut123_s[:, (ct - 1) * CB:ct * CB])

            def mt_sl(ct, i):
                return mtp_s[:, ct * N + i * 128:ct * N + (i + 1) * 128]

            def akt_sl(ct, i):
                base = 0
                return prm_s[:, base + ct * N + i * 128:
                             base + ct * N + (i + 1) * 128]

            def ak2t_sl(ct, i):
                base = 2 * N
                return prm_s[:, base + ct * N + i * 128:
                             base + ct * N + (i + 1) * 128]

            def ak4t_sl(ct, i):
                base = 4 * N
                return prm_s[:, base + ct * N + i * 128:
                             base + ct * N + (i + 1) * 128]

            def w1t_sl(ct, zm):
                base = 6 * N
                return prm_s[:, base + ct * N + zm * 128:
                             base + ct * N + (zm + 1) * 128]

            def tt_sl(ct, r):
                return tp_s[:, ct * KS + r * 128:ct * KS + (r + 1) * 128]

            def pt_sl(ct, r):
                base = 4 * KS
                return tp_s[:, base + ct * KS + r * 128:
                            base + ct * KS + (r + 1) * 128]

            def xt_sl(r, h):
                t_ = xta_s if r < 4 else xtb_s
                rr = r % 4
                return t_[:, rr * CB + h * HALF:rr * CB + (h + 1) * HALF]

            # combined state/forcing tiles: free dim = (i, c, b), i = N-tile
            za_t = consts.tile([128, N_T * CB], DT_OP, tag="za", name="za")
            f_t = consts.tile([128, N_T * CB], DT_OP, tag="f", name="f")
            ys_s = [consts.tile([128, CB], F32, tag=f"ys{i}", name=f"ys{i}")
                    for i in range(KS_T)]
            sse_s = consts.tile([128, KS_T * NHALF], F32, tag="sse",
                                name="sse")

            za3 = za_t[:].rearrange("p (i q) -> p i q", i=N_T)
            f3 = f_t[:].rearrange("p (i q) -> p i q", i=N_T)

            def mlp():
                h_s = []
                for hm in range(HID_T):
                    ph = ps_y.tile([128, HALF], F32, tag="py", name="ph")
                    nc.tensor.matmul(ph[:, 0:BLOC],
                                     w0t_s[:, hm * 128:(hm + 1) * 128],
                                     xo_s[:], start=True, stop=True)
                    ht = consts.tile([128, BLOC], DT_OP, tag=f"h{hm}",
                                     name=f"h{hm}")
                    nc.scalar.activation(ht[:], ph[:, 0:BLOC],
                                         mybir.ActivationFunctionType.Relu,
                                         bias=bb_s[:, hm:hm + 1])
                    h_s.append(ht)
                for zm in range(N_T):
                    pz = ps_y.tile([128, HALF], F32, tag="py", name="pz")
                    for ct in range(HID_T):
                        nc.tensor.matmul(pz[:, 0:BLOC],
                                         w1t_sl(ct, zm),
                                         h_s[ct][:], start=(ct == 0),
                                         stop=(ct == HID_T - 1))
                    nc.scalar.activation(za_t[:, zm * CB:zm * CB + BLOC],
                                         pz[:, 0:BLOC],
                                         mybir.ActivationFunctionType.Identity,
                                         bias=bb_s[:, HID_T + zm:
                                                   HID_T + zm + 1])

            def f_group(i, h):
                pf = ps_y.tile([128, HALF], F32, tag="py", name="pf")
                for ct in range(KUP_T):
                    nc.tensor.matmul(
                        pf[:],
                        mt_sl(ct, i),
                        ut_sl(ct)[:, h * HALF:(h + 1) * HALF],
                        start=(ct == 0), stop=(ct == KUP_T - 1))
                nc.vector.tensor_copy(
                    f_t[:, i * CB + h * HALF:i * CB + (h + 1) * HALF],
                    pf[:])

            g_t = consts.tile([128, N_T * HALF], DT_OP, tag="g", name="g")
            g2_t = consts.tile([128, N_T * 256], DT_OP, tag="g2", name="g2")
            f4 = f_t[:].rearrange("p (i c b) -> p i c b", i=N_T, c=CN)
            za4 = za_t[:].rearrange("p (i c b) -> p i c b", i=N_T, c=CN)
            g4 = g_t[:].rearrange("p (i m b) -> p i m b", i=N_T, m=CN // 2)
            g24 = g2_t[:].rearrange("p (i m b) -> p i m b", i=N_T, m=CN // 4)

            def g1_phase():
                # G1_m = Ak @ F_{2m} + F_{2m+1},  m = 0..CN/2-1
                for i in range(N_T):
                    pg = ps_y.tile([128, HALF], F32, tag="py", name="pg")
                    for ct in range(N_T):
                        nc.tensor.matmul(
                            pg[:], akt_sl(ct, i),
                            f4[:, ct, 0:CN:2, :],
                            start=(ct == 0), stop=(ct == N_T - 1))
                    nc.vector.tensor_add(
                        g4[:, i, :, :], pg[:].rearrange(
                            "p (m b) -> p m b", m=CN // 2),
                        f4[:, i, 1:CN:2, :])

            def g2_phase():
                # G2_m = Ak^2 @ G1_{2m} + G1_{2m+1},  m = 0..CN/4-1
                for i in range(N_T):
                    pg = ps_y.tile([128, HALF], F32, tag="py", name="pg2")
                    for ct in range(N_T):
                        nc.tensor.matmul(
                            pg[:, 0:256], ak2t_sl(ct, i),
                            g4[:, ct, 0:CN // 2:2, :],
                            start=(ct == 0), stop=(ct == N_T - 1))
                    nc.vector.tensor_add(
                        g24[:, i, :, :], pg[:, 0:256].rearrange(
                            "p (m b) -> p m b", m=CN // 4),
                        g4[:, i, 1:CN // 2:2, :])

            def chain_step4(m):
                # Z_{4m+4} = Ak^4 Z_{4m} + G2_m
                pb = ps_b.tile([128, N_T * BLOC], F32, tag="pb", name="pb")
                for i in range(N_T):
                    for ct in range(N_T):
                        nc.tensor.matmul(
                            pb[:, i * BLOC:(i + 1) * BLOC],
                            ak4t_sl(ct, i),
                            za4[:, ct, 4 * m, :],
                            start=(ct == 0), stop=(ct == N_T - 1))
                pb3 = pb[:].rearrange("p (i b) -> p i b", i=N_T)
                nc.vector.tensor_add(
                    za4[:, :, 4 * m + 4, :], pb3[:],
                    g24[:, :, m, :])

            def dswA(half):
                # Z_{4m+2} = Ak^2 Z_{4m} + G1_{2m}, 4 states per half
                mlo = half * 4
                for i in range(N_T):
                    pd = ps_y.tile([128, HALF], F32, tag="py", name="pdA")
                    for ct in range(N_T):
                        nc.tensor.matmul(
                            pd[:, 0:4 * BLOC], ak2t_sl(ct, i),
                            za4[:, ct, 4 * mlo:4 * (mlo + 4):4, :],
                            start=(ct == 0), stop=(ct == N_T - 1))
                    pd3 = pd[:, 0:4 * BLOC].rearrange("p (m b) -> p m b", m=4)
                    nc.vector.tensor_add(
                        za4[:, i, 4 * mlo + 2:4 * (mlo + 4):4, :], pd3[:],
                        g4[:, i, 2 * mlo:2 * (mlo + 4):2, :])

            def dswB(half):
                # Z_{2m+1} = Ak Z_{2m} + F_{2m} for m in this half
                mlo = half * (CN // 4)
                for i in range(N_T):
                    pd = ps_y.tile([128, HALF], F32, tag="py", name="pd")
                    q = CN // 4  # 8 odd states per half
                    for ct in range(N_T):
                        nc.tensor.matmul(
                            pd[:, 0:q * BLOC],
                            akt_sl(ct, i),
                            za4[:, ct, 2 * mlo:2 * (mlo + q):2, :],
                            start=(ct == 0), stop=(ct == N_T - 1))
                    pd3 = pd[:, 0:q * BLOC].rearrange("p (m b) -> p m b", m=q)
                    nc.vector.tensor_add(
                        za4[:, i, 2 * mlo + 1:2 * (mlo + q):2, :], pd3[:],
                        f4[:, i, 2 * mlo:2 * (mlo + q):2, :])

            deferred_loss = []

            def loss_ops(h, r):
                dt_ = work.tile([128, HALF], F32, tag="d", name="d")
                nc.vector.tensor_sub(
                    dt_[:], ys_s[r][:, h * HALF:(h + 1) * HALF],
                    xt_sl(r, h))
                idx = h * KS_T + r
                sq = work.tile([128, HALF], F32, tag="sq", name="sq")
                nc.scalar.activation(sq[:], dt_[:],
                                     mybir.ActivationFunctionType.Square,
                                     accum_out=sse_s[:, idx:idx + 1])

            def y_tile(h, r, defer=False):
                py = ps_y.tile([128, HALF], F32, tag="py", name="py")
                first = True
                for ct in range(KUP_T):
                    if not _conv_tile_nonzero(ct, r):
                        continue
                    nc.tensor.matmul(
                        py[:], tt_sl(ct, r),
                        ut_sl(ct)[:, h * HALF:(h + 1) * HALF],
                        start=first, stop=False)
                    first = False
                for ct in range(N_T):
                    nc.tensor.matmul(
                        py[:], pt_sl(ct, r),
                        za_t[:, ct * CB + h * HALF:ct * CB + (h + 1) * HALF],
                        start=False, stop=(ct == N_T - 1))
                # ys evacuation on DVE; loss either inline or deferred
                nc.vector.tensor_copy(
                    ys_s[r][:, h * HALF:(h + 1) * HALF], py[:])
                # stream this half of ys out now
                nc.sync.dma_start(
                    ap["yst"][r][:, h * HALF:(h + 1) * HALF],
                    ys_s[r][:, h * HALF:(h + 1) * HALF])
                if defer:
                    deferred_loss.append((h, r))
                else:
                    loss_ops(h, r)

            mlp()
            f_group(0, 0)
            f_group(0, 1)
            f_group(1, 0)
            f_group(1, 1)
            g1_phase()
            g2_phase()
            # evens mod 4: Z_4, Z_8, Z_12
            for m in range(0, 3):
                chain_step4(m)
            dswA(0)               # Z_2, Z_6, Z_10, Z_14
            dswB(0)               # odd states Z_1..Z_15
            y0 = list(range(KS_T))
            y1 = list(range(KS_T))
            y_tile(0, y0.pop(0))
            y_tile(0, y0.pop(0))
            for m in range(3, CN // 4 - 1):
                chain_step4(m)
                if y0:
                    y_tile(0, y0.pop(0))
            dswA(1)               # Z_18, Z_22, Z_26, Z_30
            if y0:
                y_tile(0, y0.pop(0))
            dswB(1)               # odd states Z_17..Z_31
            for r in y0:
                y_tile(0, r)
            for k, r in enumerate(y1):
                y_tile(1, r, defer=(k >= KS_T - 2))
            for h_, r_ in deferred_loss:
                loss_ops(h_, r_)

            # ---- outputs ----
            nc.sync.dma_start(ap["zat"][:], za_t[:])
            nc.sync.dma_start(ap["sse"][:], sse_s[:])

    nc.compile()
    return nc


_PROGRAM_CACHE = {}


def _get_program():
    key = (K, USE_F32R)
    if key not in _PROGRAM_CACHE:
        _PROGRAM_CACHE[key] = _build_program()
    return _PROGRAM_CACHE[key]


def kernel(x, u, A, Bm, Cm, W0, b0, W1, b1):
    global LAST_RESULT
    x = np.ascontiguousarray(x, np.float32)
    u = np.ascontiguousarray(u, np.float32)
    A = np.ascontiguousarray(A, np.float32)
    Bm = np.ascontiguousarray(Bm, np.float32)
    Cm = np.ascontiguousarray(Cm, np.float32)
    W0 = np.ascontiguousarray(W0, np.float32)
    b0 = np.ascontiguousarray(b0, np.float32)
    W1 = np.ascontiguousarray(W1, np.float32)
    b1 = np.ascontiguousarray(b1, np.float32)

    ops = _precompute_operators(A, Bm, Cm)
    mtp = np.ascontiguousarray(np.concatenate(
        [ops["mt"][i] for i in range(KUP_T)], axis=1), np.float32)
    prm = np.ascontiguousarray(np.concatenate(
        [ops["akt"][i] for i in range(N_T)]
        + [ops["ak2t"][i] for i in range(N_T)]
        + [ops["ak4t"][i] for i in range(N_T)]
        + [np.ascontiguousarray(W1.T.reshape(HID_T, 128, N))[i]
           for i in range(HID_T)], axis=1), np.float32)     # [128, 10N]
    tp = np.ascontiguousarray(np.concatenate(
        [ops["tt"][i] for i in range(KUP_T)]
        + [ops["pt"][i] for i in range(N_T)], axis=1), np.float32)
    w0t = np.ascontiguousarray(W0.T)                          # [S, HID]
    bb = np.ascontiguousarray(np.concatenate(
        [b0.reshape(HID_T, 128).T, b1.reshape(N_T, 128).T], axis=1),
        np.float32)                                           # [128, 6]

    in_maps = []
    for core in range(NCORES):
        bsl = slice(core * BLOC, (core + 1) * BLOC)
        # u slice -> [(j,p), (c,b)] tiles
        uu = u[bsl, O - 1:O - 1 + L, :]                       # [BLOC, L, UP]
        ut = uu.reshape(BLOC, CN, K, UP).transpose(2, 3, 1, 0)
        ut = ut.reshape(KUP_T, 128, CB)
        ut0 = np.ascontiguousarray(ut[0], np.float32)
        ut123 = np.ascontiguousarray(
            np.concatenate([ut[1], ut[2], ut[3]], axis=1), np.float32)
        # x target slice -> [(j,s), (c,b)] tiles
        xx = x[bsl, O:O + L, :]                               # [BLOC, L, S]
        xt = xx.reshape(BLOC, CN, K, S).transpose(2, 3, 1, 0)
        xt = xt.reshape(KS_T, 128, CB)
        xta = np.ascontiguousarray(
            np.concatenate([xt[r] for r in range(4)], axis=1), np.float32)
        xtb = np.ascontiguousarray(
            np.concatenate([xt[r] for r in range(4, 8)], axis=1), np.float32)
        xo = np.ascontiguousarray(x[bsl, O - 1, :].T)         # [S, BLOC]
        in_maps.append({
            "ut0": ut0, "ut123": ut123, "xta": xta, "xtb": xtb, "xo": xo,
            "mtp": mtp, "prm": prm, "tp": tp, "w0t": w0t, "bb": bb,
        })

    nc = _get_program()
    res = run_bass_kernel_spmd(nc, in_maps, core_ids=list(range(NCORES)),
                               trace=TRACE)
    LAST_RESULT = res

    # ---- reassemble full outputs ----
    ys = np.empty((L, B_SZ, S), np.float32)
    Zfull = np.empty((CN, B_SZ, N), np.float32)
    for core in range(NCORES):
        bsl = slice(core * BLOC, (core + 1) * BLOC)
        arr = res.results[core]["yst"]                        # [KS_T,128,CB]
        a = arr.reshape(K, S, CN, BLOC).transpose(2, 0, 3, 1)  # (c,j,b,s)
        ys[:, bsl, :] = a.reshape(L, BLOC, S)
        za = res.results[core]["zat"].reshape(128, N_T, CN, BLOC)
        Zfull[:, bsl, :] = np.transpose(za, (2, 3, 1, 0)).reshape(CN, BLOC, N)

    # ---- host overflow-tail patch: reproduce sequential-f32 overflow ----
    with np.errstate(all="ignore"):
        step_ok = np.isfinite(ys).all(axis=(1, 2))
        step_max = np.max(np.abs(np.nan_to_num(ys, nan=np.inf,
                                               posinf=np.inf,
                                               neginf=np.inf)), axis=(1, 2))
        danger = (~step_ok) | (step_max > np.float32(1e35))
        if danger.any():
            t_first = int(np.argmax(danger))
            c0 = max(t_first // K - 1, 0)
            z = Zfull[c0].copy()
            uu_full = u[:, O - 1:O - 1 + L, :]
            t = c0 * K
            while t < L:
                z = (z @ A.T + uu_full[:, t, :] @ Bm.T).astype(np.float32)
                ys[t] = z @ Cm.T
                t += 1
                if not np.isfinite(z).any():
                    break
            if t < L:
                ys[t:] = np.nan

        # loss = mean over steps of per-step mean  (== global mean)
        x_tgt = np.transpose(x[:, O:O + L, :], (1, 0, 2))
        loss = np.float32(np.mean((ys.astype(np.float64)
                                   - x_tgt.astype(np.float64)) ** 2))
    return loss, ys


if __name__ == "__main__":
    rng = np.random.default_rng(0)
    ins = {
        "x": rng.standard_normal((B_SZ, O + L, S)).astype(np.float32),
        "u": rng.standard_normal((B_SZ, O - 1 + L, UP)).astype(np.float32),
        "A": (0.01 * rng.standard_normal((N, N))
              + np.eye(N)).astype(np.float32),
        "Bm": (0.01 * rng.standard_normal((N, UP))).astype(np.float32),
        "Cm": (0.01 * rng.standard_normal((S, N))).astype(np.float32),
        "W0": rng.uniform(-1, 1, (HID, S)).astype(np.float32) / np.sqrt(S),
        "b0": rng.uniform(-1, 1, (HID,)).astype(np.float32) / np.sqrt(S),
        "W1": rng.uniform(-1, 1, (N, HID)).astype(np.float32) / np.sqrt(HID),
        "b1": rng.uniform(-1, 1, (N,)).astype(np.float32) / np.sqrt(HID),
    }
    loss, ys = kernel(**ins)
    print("loss", loss, "ys", ys.shape, "finite", np.isfinite(ys).mean())


# revision 29
# speedup vs baseline: 1.1997x; 1.1997x over previous
"""Deep-Koopman-operator kernel for 8 Trainium2 NeuronCores.

Math: z0 = MLP(x[:, O-1]); for t in [0,L): z <- A z + B u_t ; y_t = C z ;
loss = mean_t mean_{B,S} (y_t - x_tgt_t)^2.  Returns (loss, ys[L,B,S]).

Strategy: data-parallel over batch (B=256 -> 32 per core).  The linear
recurrence is reformulated as a chunked scan with chunk length K:
  boundary states:  Z_{c+1} = A^K Z_c + M @ u_chunk_c      (small, sequential)
  outputs:          y_{cK+j} = (C A^j) Z_c + sum_i (C A^{j-i} B) u_{cK+i}
which turns almost all work into large matmuls (P = stacked C A^j,
T = block-Toeplitz of C A^d B).  Operators are precomputed on host in f64
from the passed-in A/B/C params (param preprocessing, like weight layout
transforms).  Matmuls run in float32r (full-rate fp32 mode, ~1e-4 rounding);
measured end-to-end scale-relative error vs the f32 reference is ~3e-3.

The reference recurrence diverges (spectral radius of A ~ 1.15), so f32
overflows around t~600 and the reference output has an inf/NaN tail.  The
device computes everything; the host then re-runs only the overflow
transition window with the exact sequential-f32 semantics of the reference
so the NaN onset pattern matches, and fills NaN beyond (which is what the
sequential recurrence produces once the state is non-finite).
"""

import os

import numpy as np

import concourse.bacc as bacc
import concourse.mybir as mybir
import concourse.tile as tile
from concourse.bass_utils import run_bass_kernel_spmd

# Problem dims (hardcoded per contract)
B_SZ, O, L, N, S, HID, UP = 256, 8, 1024, 256, 32, 512, 16
NCORES = 8
BLOC = B_SZ // NCORES  # 32 batch rows per core
K = 32                 # chunk length
CN = L // K            # 32 chunks
KUP = K * UP           # 512
KS = K * S             # 1024
CB = CN * BLOC         # 1024 streaming columns (chunk-major, batch-minor)
F32 = mybir.dt.float32
F32R = mybir.dt.float32r

N_T = N // 128         # 2
KUP_T = KUP // 128     # 4
KS_T = KS // 128       # 8
HID_T = HID // 128     # 4
HALF = 512             # stream split (one PSUM bank of f32)
NHALF = CB // HALF     # 2

USE_F32R = os.environ.get("DKO_F32R", "1") == "1"
DT_OP = F32R if USE_F32R else F32  # dtype of matmul operand tensors
TRACE = os.environ.get("DKO_TRACE", "0") == "1"

# keep results of the last run for test harness inspection
LAST_RESULT = None


def _conv_tile_nonzero(ct, r):
    # T.T tile (ct, r) nonzero iff exists j >= i with j in [4r+1,4r+4],
    # i in [8ct+1, 8ct+8]  <=>  4r+4 >= 8ct+1
    return 4 * r + 4 >= 8 * ct + 1


def _precompute_operators(A, Bm, Cm):
    """Host f64 precompute of chunk operators, returned as f32 lhsT layouts."""
    A64 = A.astype(np.float64)
    B64 = Bm.astype(np.float64)
    C64 = Cm.astype(np.float64)
    Apow = [np.eye(N)]
    for _ in range(K):
        Apow.append(A64 @ Apow[-1])
    # P = stack_j C A^j (j=1..K)  [K*S, N]
    P = np.concatenate([C64 @ Apow[j] for j in range(1, K + 1)], axis=0)
    Ak = Apow[K]
    # M = [A^{K-1} B, ..., A^0 B]  [N, K*UP]
    M = np.concatenate([Apow[K - 1 - i] @ B64 for i in range(K)], axis=1)
    # T block lower triangular of Q_d = C A^d B  [K*S, K*UP]
    Q = [C64 @ Apow[d] @ B64 for d in range(K)]
    T = np.zeros((KS, KUP), np.float64)
    for j in range(1, K + 1):
        for i in range(1, j + 1):
            T[(j - 1) * S:j * S, (i - 1) * UP:i * UP] = Q[j - i]
    Ak2 = Ak @ Ak
    Ak4 = Ak2 @ Ak2
    out = {
        "akt": np.ascontiguousarray(
            Ak.T.astype(np.float32).reshape(N_T, 128, N)),
        "ak2t": np.ascontiguousarray(
            Ak2.T.astype(np.float32).reshape(N_T, 128, N)),
        "ak4t": np.ascontiguousarray(
            Ak4.T.astype(np.float32).reshape(N_T, 128, N)),
        "mt": np.ascontiguousarray(
            M.T.astype(np.float32).reshape(KUP_T, 128, N)),
        "pt": np.ascontiguousarray(
            P.T.astype(np.float32).reshape(N_T, 128, KS)),
        "tt": np.ascontiguousarray(
            T.T.astype(np.float32).reshape(KUP_T, 128, KS)),
    }
    return out


def _build_program():
    """Build the Bass/Tile SPMD program (same program on all 8 cores)."""
    nc = bacc.Bacc(trn_type="TRN2", target_bir_lowering=False, debug=False,
                   num_devices=NCORES)

    d = {}
    d["ut0"] = nc.dram_tensor("ut0", [128, CB], DT_OP, kind="ExternalInput")
    d["mtp"] = nc.dram_tensor("mtp", [128, 4 * N], DT_OP,
                              kind="ExternalInput")
    d["prm"] = nc.dram_tensor("prm", [128, 10 * N], DT_OP,
                              kind="ExternalInput")
    d["w0t"] = nc.dram_tensor("w0t", [S, HID], DT_OP, kind="ExternalInput")
    d["xo"] = nc.dram_tensor("xo", [S, BLOC], DT_OP, kind="ExternalInput")
    d["bb"] = nc.dram_tensor("bb", [128, HID_T + N_T], F32,
                             kind="ExternalInput")
    d["ut123"] = nc.dram_tensor("ut123", [128, 3 * CB], DT_OP,
                                kind="ExternalInput")
    d["tp"] = nc.dram_tensor("tp", [128, 6 * KS], DT_OP,
                             kind="ExternalInput")
    d["xta"] = nc.dram_tensor("xta", [128, 4 * CB], F32,
                              kind="ExternalInput")
    d["xtb"] = nc.dram_tensor("xtb", [128, 4 * CB], F32,
                              kind="ExternalInput")
    d["yst"] = nc.dram_tensor("yst", [KS_T, 128, CB], F32,
                              kind="ExternalOutput")
    d["zat"] = nc.dram_tensor("zat", [128, N_T * CB], DT_OP,
                              kind="ExternalOutput")
    d["sse"] = nc.dram_tensor("sse", [128, KS_T * NHALF], F32,
                              kind="ExternalOutput")

    ap = {k: v.ap() for k, v in d.items()}

    with tile.TileContext(nc) as tc:
        from contextlib import ExitStack
        with ExitStack() as ctx:
            consts = ctx.enter_context(tc.tile_pool(name="consts", bufs=1))
            work = ctx.enter_context(tc.tile_pool(name="work", bufs=3))
            ps_b = ctx.enter_context(
                tc.tile_pool(name="ps_b", bufs=2, space="PSUM"))
            ps_y = ctx.enter_context(
                tc.tile_pool(name="ps_y", bufs=6, space="PSUM"))

            # ---- persistent SBUF tiles + input DMAs (few, large) ----
            mtp_s = consts.tile([128, 4 * N], DT_OP, tag="mtp", name="mtp")
            nc.sync.dma_start(mtp_s[:], ap["mtp"][:])
            ut0_s = consts.tile([128, CB], DT_OP, tag="ut0", name="ut0")
            nc.sync.dma_start(ut0_s[:], ap["ut0"][:])
            w0t_s = consts.tile([S, HID], DT_OP, tag="w0t", name="w0t_s")
            nc.sync.dma_start(w0t_s[:], ap["w0t"][:])
            xo_s = consts.tile([S, BLOC], DT_OP, tag="xo", name="xo")
            nc.sync.dma_start(xo_s[:], ap["xo"][:])
            bb_s = consts.tile([128, HID_T + N_T], F32, tag="bb", name="bb")
            nc.sync.dma_start(bb_s[:], ap["bb"][:])
            ut123_s = consts.tile([128, 3 * CB], DT_OP, tag="ut123",
                                  name="ut123")
            nc.sync.dma_start(ut123_s[:], ap["ut123"][:])
            prm_s = consts.tile([128, 10 * N], DT_OP, tag="prm", name="prm")
            nc.sync.dma_start(prm_s[:], ap["prm"][:])
            tp_s = consts.tile([128, 6 * KS], DT_OP, tag="tp", name="tp")
            nc.sync.dma_start(tp_s[:], ap["tp"][:])
            xta_s = consts.tile([128, 4 * CB], F32, tag="xta", name="xta")
            nc.sync.dma_start(xta_s[:], ap["xta"][:])
            xtb_s = consts.tile([128, 4 * CB], F32, tag="xtb", name="xtb")
            nc.sync.dma_start(xtb_s[:], ap["xtb"][:])

            # slice views matching the old per-tile layout
            def ut_sl(ct):
                return (ut0_s[:] if ct == 0
                        else ut123_s[:, (ct - 1) * CB:ct * CB])

            def mt_sl(ct, i):
                return mtp_s[:, ct * N + i * 128:ct * N + (i + 1) * 128]

            def akt_sl(ct, i):
                base = 0
                return prm_s[:, base + ct * N + i * 128:
                             base + ct * N + (i + 1) * 128]

            def ak2t_sl(ct, i):
                base = 2 * N
                return prm_s[:, base + ct * N + i * 128:
                             base + ct * N + (i + 1) * 128]

            def ak4t_sl(ct, i):
                base = 4 * N
                return prm_s[:, base + ct * N + i * 128:
                             base + ct * N + (i + 1) * 128]

            def w1t_sl(ct, zm):
                base = 6 * N
                return prm_s[:, base + ct * N + zm * 128:
                             base + ct * N + (zm + 1) * 128]

            def tt_sl(ct, r):
                return tp_s[:, ct * KS + r * 128:ct * KS + (r + 1) * 128]

            def pt_sl(ct, r):
                base = 4 * KS
                return tp_s[:, base + ct * KS + r * 128:
                            base + ct * KS + (r + 1) * 128]

            def xt_sl(r, h):
                t_ = xta_s if r < 4 else xtb_s
                rr = r % 4
                return t_[:, rr * CB + h * HALF:rr * CB + (h + 1) * HALF]

            # combined state/forcing tiles: free dim = (i, c, b), i = N-tile
            za_t = consts.tile([128, N_T * CB], DT_OP, tag="za", name="za")
            f_t = consts.tile([128, N_T * CB], DT_OP, tag="f", name="f")
            ys_s = [consts.tile([128, CB], F32, tag=f"ys{i}", name=f"ys{i}")
                    for i in range(KS_T)]
            sse_s = consts.tile([128, KS_T * NHALF], F32, tag="sse",
                                name="sse")

            za3 = za_t[:].rearrange("p (i q) -> p i q", i=N_T)
            f3 = f_t[:].rearrange("p (i q) -> p i q", i=N_T)

            def mlp():
                h_s = []
                for hm in range(HID_T):
                    ph = ps_y.tile([128, HALF], F32, tag="py", name="ph")
                    nc.tensor.matmul(ph[:, 0:BLOC],
                                     w0t_s[:, hm * 128:(hm + 1) * 128],
                                     xo_s[:], start=True, stop=True)
                    ht = consts.tile([128, BLOC], DT_OP, tag=f"h{hm}",
                                     name=f"h{hm}")
                    nc.scalar.activation(ht[:], ph[:, 0:BLOC],
                                         mybir.ActivationFunctionType.Relu,
                                         bias=bb_s[:, hm:hm + 1])
                    h_s.append(ht)
                for zm in range(N_T):
                    pz = ps_y.tile([128, HALF], F32, tag="py", name="pz")
                    for ct in range(HID_T):
                        nc.tensor.matmul(pz[:, 0:BLOC],
                                         w1t_sl(ct, zm),
                                         h_s[ct][:], start=(ct == 0),
                                         stop=(ct == HID_T - 1))
                    nc.scalar.activation(za_t[:, zm * CB:zm * CB + BLOC],
                                         pz[:, 0:BLOC],
                                         mybir.ActivationFunctionType.Identity,
                                         bias=bb_s[:, HID_T + zm:
                                                   HID_T + zm + 1])

            def f_group(i, h):
                pf = ps_y.tile([128, HALF], F32, tag="py", name="pf")
                for ct in range(KUP_T):
                    nc.tensor.matmul(
                        pf[:],
                        mt_sl(ct, i),
                        ut_sl(ct)[:, h * HALF:(h + 1) * HALF],
                        start=(ct == 0), stop=(ct == KUP_T - 1))
                nc.vector.tensor_copy(
                    f_t[:, i * CB + h * HALF:i * CB + (h + 1) * HALF],
                    pf[:])

            g_t = consts.tile([128, N_T * HALF], DT_OP, tag="g", name="g")
            g2_t = consts.tile([128, N_T * 256], DT_OP, tag="g2", name="g2")
            f4 = f_t[:].rearrange("p (i c b) -> p i c b", i=N_T, c=CN)
            za4 = za_t[:].rearrange("p (i c b) -> p i c b", i=N_T, c=CN)
            g4 = g_t[:].rearrange("p (i m b) -> p i m b", i=N_T, m=CN // 2)
            g24 = g2_t[:].rearrange("p (i m b) -> p i m b", i=N_T, m=CN // 4)

            def g1_phase():
                # G1_m = Ak @ F_{2m} + F_{2m+1},  m = 0..CN/2-1
                for i in range(N_T):
                    pg = ps_y.tile([128, HALF], F32, tag="py", name="pg")
                    for ct in range(N_T):
                        nc.tensor.matmul(
                            pg[:], akt_sl(ct, i),
                            f4[:, ct, 0:CN:2, :],
                            start=(ct == 0), stop=(ct == N_T - 1))
                    nc.vector.tensor_add(
                        g4[:, i, :, :], pg[:].rearrange(
                            "p (m b) -> p m b", m=CN // 2),
                        f4[:, i, 1:CN:2, :])

            def g2_phase():
                # G2_m = Ak^2 @ G1_{2m} + G1_{2m+1},  m = 0..CN/4-1
                for i in range(N_T):
                    pg = ps_y.tile([128, HALF], F32, tag="py", name="pg2")
                    for ct in range(N_T):
                        nc.tensor.matmul(
                            pg[:, 0:256], ak2t_sl(ct, i),
                            g4[:, ct, 0:CN // 2:2, :],
                            start=(ct == 0), stop=(ct == N_T - 1))
                    nc.vector.tensor_add(
                        g24[:, i, :, :], pg[:, 0:256].rearrange(
                            "p (m b) -> p m b", m=CN // 4),
                        g4[:, i, 1:CN // 2:2, :])

            def chain_step4(m):
                # Z_{4m+4} = Ak^4 Z_{4m} + G2_m
                pb = ps_b.tile([128, N_T * BLOC], F32, tag="pb", name="pb")
                for i in range(N_T):
                    for ct in range(N_T):
                        nc.tensor.matmul(
                            pb[:, i * BLOC:(i + 1) * BLOC],
                            ak4t_sl(ct, i),
                            za4[:, ct, 4 * m, :],
                            start=(ct == 0), stop=(ct == N_T - 1))
                pb3 = pb[:].rearrange("p (i b) -> p i b", i=N_T)
                nc.vector.tensor_add(
                    za4[:, :, 4 * m + 4, :], pb3[:],
                    g24[:, :, m, :])

            def dswA(half):
                # Z_{4m+2} = Ak^2 Z_{4m} + G1_{2m}, 4 states per half
                mlo = half * 4
                for i in range(N_T):
                    pd = ps_y.tile([128, HALF], F32, tag="py", name="pdA")
                    for ct in range(N_T):
                        nc.tensor.matmul(
                            pd[:, 0:4 * BLOC], ak2t_sl(ct, i),
                            za4[:, ct, 4 * mlo:4 * (mlo + 4):4, :],
                            start=(ct == 0), stop=(ct == N_T - 1))
                    pd3 = pd[:, 0:4 * BLOC].rearrange("p (m b) -> p m b", m=4)
                    nc.vector.tensor_add(
                        za4[:, i, 4 * mlo + 2:4 * (mlo + 4):4, :], pd3[:],
                        g4[:, i, 2 * mlo:2 * (mlo + 4):2, :])

            def dswB(half):
                # Z_{2m+1} = Ak Z_{2m} + F_{2m} for m in this half
                mlo = half * (CN // 4)
                for i in range(N_T):
                    pd = ps_y.tile([128, HALF], F32, tag="py", name="pd")
                    q = CN // 4  # 8 odd states per half
                    for ct in range(N_T):
                        nc.tensor.matmul(
                            pd[:, 0:q * BLOC],
                            akt_sl(ct, i),
                            za4[:, ct, 2 * mlo:2 * (mlo + q):2, :],
                            start=(ct == 0), stop=(ct == N_T - 1))
                    pd3 = pd[:, 0:q * BLOC].rearrange("p (m b) -> p m b", m=q)
                    nc.vector.tensor_add(
                        za4[:, i, 2 * mlo + 1:2 * (mlo + q):2, :], pd3[:],
                        f4[:, i, 2 * mlo:2 * (mlo + q):2, :])

            deferred_loss = []

            def loss_ops(h, r):
                dt_ = work.tile([128, HALF], F32, tag="d", name="d")
                nc.vector.tensor_sub(
                    dt_[:], ys_s[r][:, h * HALF:(h + 1) * HALF],
                    xt_sl(r, h))
                idx = h * KS_T + r
                sq = work.tile([128, HALF], F32, tag="sq", name="sq")
                nc.scalar.activation(sq[:], dt_[:],
                                     mybir.ActivationFunctionType.Square,
                                     accum_out=sse_s[:, idx:idx + 1])

            def y_tile(h, r, defer=False):
                py = ps_y.tile([128, HALF], F32, tag="py", name="py")
                first = True
                for ct in range(KUP_T):
                    if not _conv_tile_nonzero(ct, r):
                        continue
                    nc.tensor.matmul(
                        py[:], tt_sl(ct, r),
                        ut_sl(ct)[:, h * HALF:(h + 1) * HALF],
                        start=first, stop=False)
                    first = False
                for ct in range(N_T):
                    nc.tensor.matmul(
                        py[:], pt_sl(ct, r),
                        za_t[:, ct * CB + h * HALF:ct * CB + (h + 1) * HALF],
                        start=False, stop=(ct == N_T - 1))
                # ys evacuation on DVE; loss either inline or deferred
                nc.vector.tensor_copy(
                    ys_s[r][:, h * HALF:(h + 1) * HALF], py[:])
                # stream this half of ys out now
                nc.sync.dma_start(
                    ap["yst"][r][:, h * HALF:(h + 1) * HALF],
                    ys_s[r][:, h * HALF:(h + 1) * HALF])
                if defer:
                    deferred_loss.append((h, r))
                else:
                    loss_ops(h, r)

            mlp()
            f_group(0, 0)
            f_group(0, 1)
            f_group(1, 0)
            f_group(1, 1)
            g1_phase()
            g2_phase()
            # evens mod 4: Z_4, Z_8, Z_12
            for m in range(0, 3):
                chain_step4(m)
            dswA(0)               # Z_2, Z_6, Z_10, Z_14
            dswB(0)               # odd states Z_1..Z_15
            y0 = list(range(KS_T))
            y1 = list(range(KS_T))
            y_tile(0, y0.pop(0))
            y_tile(0, y0.pop(0))
            for m in range(3, CN // 4 - 1):
                chain_step4(m)
                if y0:
                    y_tile(0, y0.pop(0))
            dswA(1)               # Z_18, Z_22, Z_26, Z_30
            if y0:
                y_tile(0, y0.pop(0))
            dswB(1)               # odd states Z_17..Z_31
            for r in y0:
                y_tile(0, r)
            for k, r in enumerate(y1):
                y_tile(1, r, defer=(k >= KS_T - 2))
            for h_, r_ in deferred_loss:
                loss_ops(h_, r_)

            # ---- outputs ----
            nc.sync.dma_start(ap["zat"][:], za_t[:])
            nc.sync.dma_start(ap["sse"][:], sse_s[:])

    nc.compile()
    return nc


_PROGRAM_CACHE = {}


def _get_program():
    key = (K, USE_F32R)
    if key not in _PROGRAM_CACHE:
        _PROGRAM_CACHE[key] = _build_program()
    return _PROGRAM_CACHE[key]


def kernel(x, u, A, Bm, Cm, W0, b0, W1, b1):
    global LAST_RESULT
    x = np.ascontiguousarray(x, np.float32)
    u = np.ascontiguousarray(u, np.float32)
    A = np.ascontiguousarray(A, np.float32)
    Bm = np.ascontiguousarray(Bm, np.float32)
    Cm = np.ascontiguousarray(Cm, np.float32)
    W0 = np.ascontiguousarray(W0, np.float32)
    b0 = np.ascontiguousarray(b0, np.float32)
    W1 = np.ascontiguousarray(W1, np.float32)
    b1 = np.ascontiguousarray(b1, np.float32)

    ops = _precompute_operators(A, Bm, Cm)
    mtp = np.ascontiguousarray(np.concatenate(
        [ops["mt"][i] for i in range(KUP_T)], axis=1), np.float32)
    prm = np.ascontiguousarray(np.concatenate(
        [ops["akt"][i] for i in range(N_T)]
        + [ops["ak2t"][i] for i in range(N_T)]
        + [ops["ak4t"][i] for i in range(N_T)]
        + [np.ascontiguousarray(W1.T.reshape(HID_T, 128, N))[i]
           for i in range(HID_T)], axis=1), np.float32)     # [128, 10N]
    tp = np.ascontiguousarray(np.concatenate(
        [ops["tt"][i] for i in range(KUP_T)]
        + [ops["pt"][i] for i in range(N_T)], axis=1), np.float32)
    w0t = np.ascontiguousarray(W0.T)                          # [S, HID]
    bb = np.ascontiguousarray(np.concatenate(
        [b0.reshape(HID_T, 128).T, b1.reshape(N_T, 128).T], axis=1),
        np.float32)                                           # [128, 6]

    in_maps = []
    for core in range(NCORES):
        bsl = slice(core * BLOC, (core + 1) * BLOC)
        # u slice -> [(j,p), (c,b)] tiles
        uu = u[bsl, O - 1:O - 1 + L, :]                       # [BLOC, L, UP]
        ut = uu.reshape(BLOC, CN, K, UP).transpose(2, 3, 1, 0)
        ut = ut.reshape(KUP_T, 128, CB)
        ut0 = np.ascontiguousarray(ut[0], np.float32)
        ut123 = np.ascontiguousarray(
            np.concatenate([ut[1], ut[2], ut[3]], axis=1), np.float32)
        # x target slice -> [(j,s), (c,b)] tiles
        xx = x[bsl, O:O + L, :]                               # [BLOC, L, S]
        xt = xx.reshape(BLOC, CN, K, S).transpose(2, 3, 1, 0)
        xt = xt.reshape(KS_T, 128, CB)
        xta = np.ascontiguousarray(
            np.concatenate([xt[r] for r in range(4)], axis=1), np.float32)
        xtb = np.ascontiguousarray(
            np.concatenate([xt[r] for r in range(4, 8)], axis=1), np.float32)
        xo = np.ascontiguousarray(x[bsl, O - 1, :].T)         # [S, BLOC]
        in_maps.append({
            "ut0": ut0, "ut123": ut123, "xta": xta, "xtb": xtb, "xo": xo,
            "mtp": mtp, "prm": prm, "tp": tp, "w0t": w0t, "bb": bb,
        })

    nc = _get_program()
    res = run_bass_kernel_spmd(nc, in_maps, core_ids=list(range(NCORES)),
                               trace=TRACE)
    LAST_RESULT = res

    # ---- reassemble full outputs ----
    ys = np.empty((L, B_SZ, S), np.float32)
    Zfull = np.empty((CN, B_SZ, N), np.float32)
    for core in range(NCORES):
        bsl = slice(core * BLOC, (core + 1) * BLOC)
        arr = res.results[core]["yst"]                        # [KS_T,128,CB]
        a = arr.reshape(K, S, CN, BLOC).transpose(2, 0, 3, 1)  # (c,j,b,s)
        ys[:, bsl, :] = a.reshape(L, BLOC, S)
        za = res.results[core]["zat"].reshape(128, N_T, CN, BLOC)
        Zfull[:, bsl, :] = np.transpose(za, (2, 3, 1, 0)).reshape(CN, BLOC, N)

    # ---- host overflow-tail patch: reproduce sequential-f32 overflow ----
    with np.errstate(all="ignore"):
        step_ok = np.isfinite(ys).all(axis=(1, 2))
        step_max = np.max(np.abs(np.nan_to_num(ys, nan=np.inf,
                                               posinf=np.inf,
                                               neginf=np.inf)), axis=(1, 2))
        danger = (~step_ok) | (step_max > np.float32(1e35))
        if danger.any():
            t_first = int(np.argmax(danger))
            c0 = max(t_first // K - 1, 0)
            z = Zfull[c0].copy()
            uu_full = u[:, O - 1:O - 1 + L, :]
            t = c0 * K
            while t < L:
                z = (z @ A.T + uu_full[:, t, :] @ Bm.T).astype(np.float32)
                ys[t] = z @ Cm.T
                t += 1
                if not np.isfinite(z).any():
                    break
            if t < L:
                ys[t:] = np.nan

        # loss = mean over steps of per-step mean  (== global mean)
        x_tgt = np.transpose(x[:, O:O + L, :], (1, 0, 2))
        loss = np.float32(np.mean((ys.astype(np.float64)
                                   - x_tgt.astype(np.float64)) ** 2))
    return loss, ys


if __name__ == "__main__":
    rng = np.random.default_rng(0)
    ins = {
        "x": rng.standard_normal((B_SZ, O + L, S)).astype(np.float32),
        "u": rng.standard_normal((B_SZ, O - 1 + L, UP)).astype(np.float32),
        "A": (0.01 * rng.standard_normal((N, N))
              + np.eye(N)).astype(np.float32),
        "Bm": (0.01 * rng.standard_normal((N, UP))).astype(np.float32),
        "Cm": (0.01 * rng.standard_normal((S, N))).astype(np.float32),
        "W0": rng.uniform(-1, 1, (HID, S)).astype(np.float32) / np.sqrt(S),
        "b0": rng.uniform(-1, 1, (HID,)).astype(np.float32) / np.sqrt(S),
        "W1": rng.uniform(-1, 1, (N, HID)).astype(np.float32) / np.sqrt(HID),
        "b1": rng.uniform(-1, 1, (N,)).astype(np.float32) / np.sqrt(HID),
    }
    loss, ys = kernel(**ins)
    print("loss", loss, "ys", ys.shape, "finite", np.isfinite(ys).mean())
